# revision 38
# baseline (speedup 1.0000x reference)
"""Trainium2 Bass kernel for PixelContrastLoss._contrastive (supervised
prototype contrastive loss).

Key algebraic reduction: the reference builds an [N, N] affinity matrix
(N = A*n_view = 5120) between contrast features and per-anchor prototype
rows, but the prototype side has only NUM_CLASSES = 19 distinct rows
(prototypes[labels]).  Every per-row quantity (row max, masked exp sums,
positive-pair log prob) therefore collapses onto the [N, 19] matrix
S = X @ (P / T)^T with per-class column multiplicities w[c] =
n_view * count(labels == c):

  m_i    = max_{c present} (S[i, c] + ln w[c])          (stabilizer)
  neg_i  = sum_{c != l_i} w[c] * exp(S[i, c] - m_i)
  lp_i   = (S[i, l_i] - m_i) - log(exp(S[i, l_i] - m_i) + neg_i)
  loss   = mean_i( -(T/BT) * [npos_i > 0] * lp_i )      (npos ratio == 1 in f32)

(The max-subtraction cancels exactly, so any per-row stabilizer is valid;
using max(S + ln w) keeps every exponent <= 0.)

The reference additionally NaN-guards the loss: when some row's
neg_logits underflows to exactly 0 in float32, log(0) = -inf appears in
masked-out columns and 0 * -inf = NaN poisons the mean; the reference
then returns 0.0.  That happens iff for some row every other-class
centered logit is below the f32 exp underflow threshold, i.e.
S[i, l_i] - max_{c != l_i, present} S[i, c] > TAU.  The kernel detects
this on-device with a relu-based poison term folded into the partial
sums; the host maps a huge partial back to 0.0.

Sharding: the 5120 rows (256 anchors x 20 views) are data-parallel; each
of the 8 cores gets 32 anchors = 640 rows.  Prototypes and the per-class
vectors are replicated.  Each core emits a [128,1] per-partition partial
sum; the host adds 8 x 128 floats (the "all-reduce" of a scalar).

Layout/perf notes:
 - matmul inputs (X^T and (P/T)^T, both contraction-major) are bf16:
   halves the dominant DMA and is far inside the loss tolerance
   (S error ~0.3 abs on a +-700 range).
 - everything rides ONE bf16 DRAM tensor [k0 | k1 | meta]; the f32 side
   data is bit-packed as bf16 pairs and viewed back with AP.bitcast.
   dma_start calls carry ~2us of fixed cost each (issue + completion
   semaphore), so the kernel issues exactly 3: planes, meta, out.
 - a single ACT table set (natural_log_exp_and_others) covers exp+ln,
   avoiding a ~1.3us mid-kernel table reload (see _patch_act_tables_once).
"""

import os
import sys

import numpy as np

for _p in ("/opt/trn_rl_repo", "/root/.axon_site/_ro/trn_rl_repo"):
    if os.path.isdir(_p) and _p not in sys.path:
        sys.path.insert(0, _p)

from contextlib import ExitStack

import ml_dtypes

import concourse.bass as bass
import concourse.tile as tile
from concourse import bacc, mybir
from concourse.bass_utils import run_bass_kernel_spmd

A, V, D, C = 256, 20, 256, 19   # anchors, views, feat dim, classes
NCORES = 8
APC = A // NCORES               # anchors per core (32)
R = APC * V                     # rows per core (640)
B = R // 128                    # 128-row blocks per core (5)
N = A * V                       # total rows (5120)
TEMP, BASE_TEMP = 0.1, 0.07
PEN = -1.0e5                    # additive mask excluding absent classes
TAU = 95.0                      # f32 exp-underflow gap threshold (87.3 FTZ .. 104 denormal)
POISON = 1.0e24                 # NaN-flag poison scale
F32 = mybir.dt.float32
BF16 = mybir.dt.bfloat16
BF16_NP = ml_dtypes.bfloat16

RC = R + C                      # packed row width per k-chunk (659)
MF = C + B * C + 2 * B          # meta width: lw | oh | lane[invw,coef] (124)

_NC_CACHE = {}


def _bcast_inner(ap: bass.AP, n: int) -> bass.AP:
    """[P, M] -> [P, M, n] with stride-0 innermost dim."""
    return bass.AP(tensor=ap.tensor, offset=ap.offset, ap=ap.ap + [[0, n]])


def _bcast_mid(ap: bass.AP, n: int) -> bass.AP:
    """[P, M] -> [P, n, M] with stride-0 middle dim."""
    return bass.AP(tensor=ap.tensor, offset=ap.offset, ap=[ap.ap[0], [0, n], ap.ap[1]])


def _patch_act_tables_once():
    """Constrain the ACT-table chooser to the one set that holds every
    function this kernel uses (exp, ln, relu, copy) so only a single
    LoadActFuncSet is emitted — the greedy per-function choice otherwise
    inserts a ~1.3us Exp->Ln table switch on the critical path.  Entries
    are blanked in place (not removed) so act_func_set_id indices keep
    matching act_info.json."""
    if _NC_CACHE.get("act_patched"):
        return
    orig = bacc.get_activation_tables

    def patched(module_arch):
        tables = orig(module_arch)
        want = {"natural_log_exp_and_others"}
        if want <= set(tables):
            tables = {name: (funcs if name in want else set())
                      for name, funcs in tables.items()}
        return tables

    bacc.get_activation_tables = patched
    _NC_CACHE["act_patched"] = True


def _build_bass() -> bass.Bass:
    # Bacc (not raw Bass): its compile() legalizes sync waits — TRN2 allows
    # at most one semaphore wait per instruction, Tile can emit more.
    _patch_act_tables_once()
    nc = bacc.Bacc("TRN2", target_bir_lowering=False, debug=False)
    inp = nc.dram_tensor("inp", [128, 2 * RC + 2 * MF], BF16,
                         kind="ExternalInput").ap()
    out = nc.dram_tensor("out", [128, 1], F32, kind="ExternalOutput").ap()

    AT = mybir.ActivationFunctionType
    OP = mybir.AluOpType
    AX = mybir.AxisListType

    with tile.TileContext(nc) as tc, ExitStack() as ctx:
        consts = ctx.enter_context(tc.tile_pool(name="consts", bufs=1))
        psum = ctx.enter_context(tc.tile_pool(name="psum", bufs=1, space="PSUM"))
        work = ctx.enter_context(tc.tile_pool(name="work", bufs=1))

        # Two DMAs, layout [k0 | k1 | meta]: the first carries both matmul
        # planes (the PSUM accumulation-group order needs all of them before
        # the last matmul anyway); the small f32 side data (bit-packed into
        # the bf16 tensor) follows on the gpsimd SWDGE ring so the two
        # issue slices run in parallel and DVE's constants arrive before
        # the matmuls finish.
        inpt = consts.tile([128, 2 * RC + 2 * MF], BF16, tag="inpt")
        nc.sync.dma_start(inpt[:, 0:2 * RC], inp[:, 0:2 * RC])
        nc.gpsimd.dma_start(inpt[:, 2 * RC:], inp[:, 2 * RC:])
        xt0 = inpt[:, 0:RC]
        xt1 = inpt[:, RC:2 * RC]
        metat = inpt[:, 2 * RC:].bitcast(F32)               # [128, MF]
        assert tuple(metat.shape) == (128, MF), metat.shape
        lwt = metat[:, 0:C]
        oht = metat[:, C:C + B * C].rearrange("p (b c) -> p b c", c=C)
        lanet = metat[:, C + B * C:MF].rearrange("p (k b) -> p k b", b=B)
        zerot = consts.tile([128, 1], F32, tag="zerot")
        nc.vector.memset(zerot, 0.0)

        # S = X @ (P/T)^T for all 5 row-blocks into one PSUM tile [128, B, C].
        # b outer: one accumulation group open at a time (PSUM zero region
        # = the whole bank, so groups in one tile must not interleave).
        ps95 = psum.tile([128, B, C], F32, tag="ps95")
        for b in range(B):
            for k, xk in enumerate((xt0, xt1)):
                nc.tensor.matmul(ps95[:, b, :],
                                 lhsT=xk[:, b * 128:(b + 1) * 128],
                                 rhs=xk[:, R:RC],
                                 start=(k == 0), stop=(k == 1))

        # Batched epilogue over the [128, B, C] block.
        sl95 = work.tile([128, B, C], F32, tag="sl95")      # S + ln w (+pen)
        nc.vector.tensor_add(sl95, ps95, _bcast_mid(lwt, B))
        negm = work.tile([128, B], F32, tag="negm")         # -max(S + lw)
        nc.vector.reduce_max(negm, sl95, axis=AX.X, negate=True)
        sm95 = work.tile([128, B, C], F32, tag="sm95")      # centered
        nc.vector.tensor_add(sm95, sl95, _bcast_inner(negm[:, :], C))
        ew95 = work.tile([128, B, C], F32, tag="ew95")      # w_c e_c (scaled)
        nc.scalar.activation(ew95, sm95, AT.Exp, bias=zerot, scale=1.0)
        negf = work.tile([128, B], F32, tag="negf")         # sum_c w_c e_c
        nc.vector.reduce_sum(negf, ew95, axis=AX.X)
        t2 = work.tile([128, B, C], F32, tag="t2")
        nc.vector.tensor_mul(t2, sm95, oht)
        zown = work.tile([128, B], F32, tag="zown")         # S_own + ln w_own - m
        nc.vector.reduce_sum(zown, t2, axis=AX.X)
        ewown = work.tile([128, B], F32, tag="ewown")       # w_own e_own
        nc.scalar.activation(ewown, zown, AT.Exp, bias=zerot, scale=1.0)

        # denominator: e_own + (negf - ewown) = negf + ewown*(invw - 1)
        tm5 = work.tile([128, B], F32, tag="tm5")
        nc.vector.tensor_mul(tm5, ewown, lanet[:, 0, :])    # lane0 = invw - 1
        dn5 = work.tile([128, B], F32, tag="dn5")
        nc.vector.tensor_add(dn5, negf, tm5)
        # NaN-guard flag: reference NaN <=> some row's neg sum is exactly 0
        # in f32 (all other-class exps underflowed / absorbed), i.e.
        # negf == ewown.  1/0 indicator, scaled by POISON in the final op.
        pz5 = work.tile([128, B], F32, tag="pz5")
        nc.vector.tensor_tensor(pz5, negf, ewown, op=OP.is_equal)
        ln5 = work.tile([128, B], F32, tag="ln5")
        nc.scalar.activation(ln5, dn5, AT.Ln, bias=zerot, scale=1.0)
        lp5 = work.tile([128, B], F32, tag="lp5")           # log prob + ln w_own
        nc.vector.tensor_sub(lp5, zown, ln5)
        r5 = work.tile([128, B], F32, tag="r5")             # coef * (lp + ln w_own)
        nc.vector.tensor_mul(r5, lp5, lanet[:, 1, :])
        # r2 = r5 + POISON*flag, racc = per-partition sum, in one instruction
        r2 = work.tile([128, B], F32, tag="r2")
        racc = work.tile([128, 1], F32, tag="racc")
        nc.vector.scalar_tensor_tensor(r2, pz5, POISON, r5, OP.mult, OP.add,
                                       accum_out=racc)
        nc.sync.dma_start(out, racc)
    nc.compile()
    return nc


def _get_nc() -> bass.Bass:
    nc = _NC_CACHE.get("nc")
    if nc is None:
        nc = _build_bass()
        _NC_CACHE["nc"] = nc
    return nc


def _prepare_in_maps(feats, P, labels, btch):
    cnt = np.bincount(labels, minlength=C)
    present = cnt > 0
    w = (V * cnt).astype(np.float64)
    lwv = np.where(present, np.log(np.maximum(w, 1.0)), PEN).astype(np.float32)
    ptT = np.ascontiguousarray(P.T / np.float32(TEMP)).astype(BF16_NP)  # [D, C]
    absent = (~present).astype(np.float32)

    in_maps = []
    hostc = 0.0
    for k in range(NCORES):
        asl = slice(k * APC, (k + 1) * APC)
        lab_k = labels[asl]
        btk = btch[asl]
        x = feats[asl].reshape(R, D)
        inp = np.empty((128, 2 * RC + 2 * MF), BF16_NP)
        xtp = inp[:, 0:2 * RC].reshape(128, 2, RC)
        xT = np.ascontiguousarray(x.T).astype(BF16_NP).reshape(2, 128, R)
        xtp[:, 0, :R] = xT[0]
        xtp[:, 1, :R] = xT[1]
        xtp[:, 0, R:] = ptT[:128]
        xtp[:, 1, R:] = ptT[128:]
        ra = np.arange(R) // V                                  # local anchor per row
        l_r = lab_k[ra]                                         # own class per row

        eq = labels[None, :] == lab_k[:, None]                  # [APC, A]
        ne = btch[None, :] != btk[:, None]
        npos_a = V * (eq & ne).sum(axis=1).astype(np.float64)   # [APC]
        npos_r = npos_a[ra]
        coef_r = ((-(TEMP / BASE_TEMP) / N)
                  * (npos_r / (npos_r + 1e-8))).astype(np.float32)

        def to_pb(v):                                           # [R] -> [128, B]
            return np.ascontiguousarray(v.reshape(B, 128).T)

        oh = np.zeros((128, B, C), np.float32)
        rb = l_r.reshape(B, 128)
        for b in range(B):
            oh[np.arange(128), b, rb[b]] = 1.0
        meta = np.empty((128, MF), np.float32)
        meta[:, 0:C] = lwv[None, :]
        meta[:, C:C + B * C] = oh.reshape(128, B * C)
        meta[:, C + B * C:C + B * C + B] = to_pb(
            (1.0 / w[l_r] - 1.0).astype(np.float32))
        meta[:, C + B * C + B:MF] = to_pb(coef_r)
        inp[:, 2 * RC:] = meta.view(BF16_NP)                # f32 bits as bf16 pairs

        # ln w_own is subtracted on the host instead of on-device:
        # sum_rows coef * ln w_own (exact, fp64)
        hostc += float(np.sum(coef_r.astype(np.float64) * lwv[l_r].astype(np.float64)))
        in_maps.append({"inp": inp})
    return in_maps, hostc


def _combine(partials, hostc):
    total = float(np.sum(np.asarray(partials, dtype=np.float64))) - hostc
    if (not np.isfinite(total)) or abs(total) > 1e12:
        total = 0.0   # reference NaN-guard path
    return np.array(total, dtype=np.float32)


def kernel(feats_, real_prototypes, labels, btch):
    feats = np.ascontiguousarray(np.asarray(feats_), dtype=np.float32)
    P = np.ascontiguousarray(np.asarray(real_prototypes), dtype=np.float32)
    labels_i = np.asarray(labels).astype(np.int64)
    btch_i = np.asarray(btch).astype(np.int64)
    assert feats.shape == (A, V, D) and P.shape == (C, D)

    nc = _get_nc()
    in_maps, hostc = _prepare_in_maps(feats, P, labels_i, btch_i)
    trace = os.environ.get("PCL_TRACE") == "1"
    res = run_bass_kernel_spmd(nc, in_maps, list(range(NCORES)), trace=trace)
    if trace and res.exec_time_ns is not None:
        print(f"HW exec time: {res.exec_time_ns} ns")
        _NC_CACHE["last_exec_time_ns"] = res.exec_time_ns
        _NC_CACHE["last_results"] = res
    partials = [float(np.sum(r["out"], dtype=np.float64)) for r in res.results]
    _NC_CACHE["last_partials"] = partials
    return _combine(partials, hostc)
